# revision 39
# baseline (speedup 1.0000x reference)
"""Trainium2 Bass kernel for sparse multi-headed attention (v3, bf16).

Semantics (verified against the reference):
  q = x_q @ Wq.T + bq (per head, dk=32), same for k, v
  for each row s: attend to keys {s-c : c in (5,3,1,0), c <= s}
    score_c[s] = q[s].k[s-c] / sqrt(4)
    p = softmax over valid offsets
    attn[s] = sum_c p_c[s] * v[s-c]
  y = attn @ Wo.T + bo

Sharding: data-parallel over d_stock (8 stocks -> 8 cores). Each core
processes 4 (stock,batch) pairs = 2048 rows. Weights replicated.

Device layout: feature-major bf16 activations [256 feats = 2 chips x 128,
rows]; fp32 PSUM accumulation. Scores/softmax live in a pair-block layout
[128 partitions = 4 pair-blocks x (8 heads + 24 unused), 4 offsets x 512].
k is stored as an alignment pair ([128, 2, 520]: row 0 = shifted by one
element, row 1 = plain) so every shifted bf16 read is 4-byte aligned and
the offset pairs (5,3) / (1,0) collapse into single strided DVE ops in
packed 2x mode. v is a single padded row ([128, 520]); its reads feed
1x PSUM-operand multiplies where alignment doesn't matter. Pair-boundary
pad columns are never consumed with nonzero probability (the score mask
zeroes those lanes), so they are just memset once — no tail writes.
Output is produced transposed (feature-major) with the bias applied
per-partition; the host transposes back.
"""

import numpy as np

from concourse import bacc, bass, mybir, tile
from concourse.bass_utils import run_bass_kernel_spmd

DS, NB, S, DM, H, DK = 8, 4, 512, 256, 8, 32
CONS = (5, 3, 1, 0)
NCORES = 8
NPAIR = NB  # pairs per core (1 stock x 4 batches)
ROWS = NPAIR * S  # 2048
P = 128
PADC = 8  # zero pad columns in front of k/v tiles
KW = PADC + S  # 520
NEG = -1e9
SCALE = 0.5  # 1/sqrt(n_att)

f32 = mybir.dt.float32
bf16 = mybir.dt.bfloat16
Act = mybir.ActivationFunctionType


def _shift_ap_k(t, step):
    """AP over a [P, 2, KW] k pair tile covering two shifted 512-reads.

    step=2  -> offsets (5,3): row 0 (shifted copy) at bases 4,6
    step=KW -> offsets (1,0): row 0 base 8, row 1 (plain) base 8
    All bases even -> 4-byte aligned -> DVE 2x packed mode.
    """
    tv = t[:].rearrange("a o k -> a (o k)")
    off = 4 if step == 2 else PADC
    sl = tv[:, off:off + S]
    return bass.AP(tensor=sl.tensor, offset=sl.offset,
                   ap=[sl.ap[0], [step, 2], [1, S]])


def _shift_ap_v(t, half):
    """AP over a [P, KW] v tile: half 0 -> offsets (5,3) at bases 3,5;
    half 1 -> offsets (1,0) at bases 7,8. Feeds 1x PSUM-operand muls."""
    off, step = (3, 2) if half == 0 else (7, 1)
    sl = t[:, off:off + S]
    return bass.AP(tensor=sl.tensor, offset=sl.offset,
                   ap=[sl.ap[0], [step, 2], [1, S]])


def _emit(ctx, tc, nc, d, y_dram):
    main = ctx.enter_context(tc.tile_pool(name="main", bufs=1))
    prodp = ctx.enter_context(tc.tile_pool(name="prodp", bufs=8))
    utp = ctx.enter_context(tc.tile_pool(name="utp", bufs=10))
    smx = ctx.enter_context(tc.tile_pool(name="smx", bufs=6))
    pj_ctx = tc.tile_pool(name="pj", bufs=2, space="PSUM")
    psum_pj = pj_ctx.__enter__()
    sc_ctx = tc.tile_pool(name="sc", bufs=1, space="PSUM")
    psum_sc = sc_ctx.__enter__()
    # scores tile allocated early: its last bank doubles as the dummy
    # (HAM keepalive) matmul target until real scores re-clear it
    sc = psum_sc.tile([P, 4 * 512], f32, name="scores")

    # ---------------- PE warmup (HAM un-throttle) while DMAs run ----
    wtile = main.tile([P, 512], bf16, name="wtile")
    nc.vector.memset(wtile[:], 0.0)

    def dummy_mm():
        nc.tensor.matmul(
            sc[:, 1536:2048], lhsT=wtile[:, 0:P], rhs=wtile[:],
            start=True, stop=True)

    for i in range(5):
        dummy_mm()

    # ---------------- loads (scalar queue carries NO DMAs) ----------
    xs = {}
    for name in ("xq", "xk", "xv"):
        for ch in range(2):
            xs[name, ch] = main.tile([P, ROWS], bf16, name=f"{name}{ch}")
    # packed weights: wqk[kch] = [wq | wk] cols, wvo[kch] = [wv | wo]
    wqk = []
    wvo = []
    for kch in range(2):
        t = main.tile([P, 2 * DM], bf16, name=f"wqk{kch}")
        eng = nc.sync if kch == 0 else nc.scalar
        eng.dma_start(out=t[:], in_=d["wqk"][kch * P:(kch + 1) * P, :])
        wqk.append(t)
        t = main.tile([P, 2 * DM], bf16, name=f"wvo{kch}")
        wvo.append(t)
    # (tile, col offset) for each packed weight
    ws = {}
    for kch in range(2):
        ws["wq", kch] = (wqk[kch], 0)
        ws["wk", kch] = (wqk[kch], DM)
        ws["wv", kch] = (wvo[kch], 0)
        ws["wo", kch] = (wvo[kch], DM)
    # biases packed [128, 8]: col ch*4+j = (bq, bk, bv, bo)[j] of chip ch
    bias_sb = main.tile([P, 8], f32, name="bias")
    nc.sync.dma_start(out=bias_sb[:], in_=d["bias"])
    selkm_sb = main.tile([P, 2 * 224], bf16, name="selkm")
    nc.sync.dma_start(out=selkm_sb[:], in_=d["selkm"])
    # critical first slices: small, issued first (on three separate
    # rings) so they get bandwidth ahead of the bulk
    for ch in range(2):
        eng = nc.sync if ch == 0 else nc.scalar
        eng.dma_start(out=xs["xq", ch][:, 0:512],
                      in_=d["xq"][ch * P:(ch + 1) * P, 0:512])
        nc.gpsimd.dma_start(out=xs["xk", ch][:, 0:512],
                            in_=d["xk"][ch * P:(ch + 1) * P, 0:512])
    selmk = main.tile([P, 8 * P], bf16, name="selmk")

    # ---------------- q/k/v projections (PE, bf16) ----------------
    # Per chip: q [128, NPAIR, 512]; k alignment-pair [128, NPAIR, 2, 520]
    # (row 0 = row 1 shifted one element); v padded row [128, NPAIR, 520].
    # Projections are emitted in pair-pairs: 4 matmuls into a 2-bank PSUM
    # tile drained by a single 1024-col scalar activation (halves the
    # activation count on the pacing scalar queue). Pad heads are memset
    # once -- masked lanes never consume them with nonzero probability.
    q = {}
    kvv = {}
    vv = {}
    for ch in range(2):
        q[ch] = main.tile([P, NPAIR, 512], bf16, name=f"q{ch}")
        kvv[ch] = main.tile([P, NPAIR, 2, KW], bf16, name=f"k{ch}")
        vv[ch] = main.tile([P, NPAIR, KW], bf16, name=f"v{ch}")
    # memsets sit between the critical and bulk DMA issues on the gpsimd
    # queue, giving the critical transfers a bandwidth head start
    for ch in range(2):
        for p in range(NPAIR):
            nc.gpsimd.memset(kvv[ch][:, p, 0, 0:PADC + 1], 0.0)
            nc.gpsimd.memset(kvv[ch][:, p, 1, 0:PADC], 0.0)
            nc.gpsimd.memset(vv[ch][:, p, 0:PADC], 0.0)
    for ch in range(2):
        nc.gpsimd.dma_start(out=xs["xq", ch][:, 512:2048],
                            in_=d["xq"][ch * P:(ch + 1) * P, 512:2048])
        nc.gpsimd.dma_start(out=xs["xk", ch][:, 512:2048],
                            in_=d["xk"][ch * P:(ch + 1) * P, 512:2048])
    for kch in range(2):
        nc.gpsimd.dma_start(out=wvo[kch][:],
                            in_=d["wvo"][kch * P:(kch + 1) * P, :])
    for ch in range(2):
        nc.gpsimd.dma_start(out=xs["xv", ch][:],
                            in_=d["xv"][ch * P:(ch + 1) * P, :])
    nc.gpsimd.dma_start(out=selmk[:], in_=d["selmk"])

    def _cols_ap(t, pattern, base, outer, count, inner):
        tv = t[:].rearrange(pattern)
        sl = tv[:, base:base + inner]
        return bass.AP(tensor=sl.tensor, offset=sl.offset,
                       ap=[sl.ap[0], [outer, count], [1, inner]])

    def project2(name, wname, bcol, np0, ch):
        ps = psum_pj.tile([P, 2, 512], f32, name="pjt", tag="pjt")
        for i in range(2):
            for kch in range(2):
                wt, off = ws[wname, kch]
                nc.tensor.matmul(
                    ps[:, i, :],
                    lhsT=wt[:, off + ch * P:off + (ch + 1) * P],
                    rhs=xs[name, kch][:, (np0 + i) * 512:(np0 + i + 1) * 512],
                    start=(kch == 0), stop=(kch == 1))
        bias_ap = bias_sb[:, ch * 4 + bcol:ch * 4 + bcol + 1]
        if name == "xq":
            out = q[ch][:, np0:np0 + 2, :]
        elif name == "xk":
            out = _cols_ap(kvv[ch], "a n o k -> a (n o k)",
                           np0 * 2 * KW + KW + PADC, 2 * KW, 2, 512)
        else:
            out = _cols_ap(vv[ch], "a n k -> a (n k)",
                           np0 * KW + PADC, KW, 2, 512)
        nc.scalar.activation(out, ps[:], Act.Identity, bias=bias_ap)
        if name == "xk":
            # shifted row 0 = cheap DVE SBUF copy of row 1 (both pairs)
            nc.vector.tensor_copy(
                _cols_ap(kvv[ch], "a n o k -> a (n o k)",
                         np0 * 2 * KW + PADC + 1, 2 * KW, 2, 511),
                _cols_ap(kvv[ch], "a n o k -> a (n o k)",
                         np0 * 2 * KW + KW + PADC, 2 * KW, 2, 511))

    for np0 in (0, 2):
        for ch in range(2):
            project2("xq", "wq", 0, np0, ch)
            project2("xk", "wk", 1, np0, ch)
    # v projections fill the PE gap while the first products run on DVE
    for np0 in (0, 2):
        for ch in range(2):
            project2("xv", "wv", 2, np0, ch)

    # ---------------- scores ----------------
    # sc[32*pair + h, ci*512 + s] = q_h[s] . k_h[s-c] * 0.5
    # Half-major order: regions (5,3) finish first so their mask + exp +
    # partial denominator overlap the (1,0) product/matmul batch.
    p_sb = main.tile([P, 4, 512], bf16, name="p_sb")
    d1 = smx.tile([P, 512], bf16, name="d1", tag="smx")
    for half, step in ((0, 2), (1, KW)):
        for p in range(NPAIR):
            for ch in range(2):
                q_b = q[ch][:, p, :].rearrange(
                    "a (o s) -> a o s", o=1).broadcast_to([P, 2, 512])
                pr = prodp.tile([P, 2, 512], bf16, name="prod", tag="prod")
                nc.vector.tensor_mul(
                    pr[:], q_b, _shift_ap_k(kvv[ch][:, p], step))
                for j in range(2):
                    ci = half * 2 + j
                    nc.tensor.matmul(
                        sc[:, ci * 512:(ci + 1) * 512],
                        lhsT=selkm_sb[:, ch * 224 + 96 - 32 * p:
                                      ch * 224 + 224 - 32 * p],
                        rhs=pr[:, j, :],
                        start=(p == 0 and ch == 0),
                        stop=(p == 3 and ch == 1))
        # mask scores for s_loc < c -> -1e9, then exp this half (one
        # 1024-col activation spanning both regions)
        for j in range(2):
            ci = half * 2 + j
            c = CONS[ci]
            if c:
                nc.vector.memset(sc[:, ci * 512: ci * 512 + c], NEG)
        nc.scalar.activation(
            p_sb[:, half * 2:half * 2 + 2, :],
            sc[:, half * 1024:(half + 1) * 1024].rearrange(
                "a (o s) -> a o s", o=2), Act.Exp)
        if half == 0:
            nc.vector.tensor_add(d1[:], p_sb[:, 0, :], p_sb[:, 1, :])

    # ---------------- softmax over the 4 offsets (no max-sub: scores
    # are O(15) and masked lanes exp to 0) ----------------
    dummy_mm()  # HAM keepalive across the softmax bridge
    d2 = smx.tile([P, 512], bf16, name="d2", tag="smx")
    nc.vector.tensor_add(d2[:], p_sb[:, 2, :], p_sb[:, 3, :])
    den = smx.tile([P, 512], f32, name="den", tag="smx")
    nc.vector.tensor_add(den[:], d1[:], d2[:])
    rcp = smx.tile([P, 512], f32, name="rcp", tag="smx")
    nc.vector.reciprocal_approx_fast(rcp[:], den[:])
    rcpb = smx.tile([P, 512], bf16, name="rcpb", tag="smx")
    nc.vector.tensor_copy(rcpb[:], rcp[:])
    dummy_mm()
    rcp_b = rcpb[:].rearrange("a (o s) -> a o s", o=1).broadcast_to(
        [P, 4, 512])
    nc.vector.tensor_mul(p_sb[:], p_sb[:], rcp_b)

    # free proj/score PSUM, open attention-phase pools
    sc_ctx.__exit__(None, None, None)
    pj_ctx.__exit__(None, None, None)
    psum_bc = ctx.enter_context(tc.tile_pool(name="bc", bufs=2, space="PSUM"))
    psum_y = ctx.enter_context(tc.tile_pool(name="yp", bufs=2, space="PSUM"))

    def dummy_y():
        wps = psum_y.tile([P, 512], f32, name="ypt", tag="ypt")
        nc.tensor.matmul(
            wps[:], lhsT=wtile[:, 0:P], rhs=wtile[:], start=True, stop=True)

    # ---------------- attention + output projection, per pair --------
    y_sb = [main.tile([P, NPAIR, 512], bf16, name=f"y{o}") for o in range(2)]

    def emit_y(p, usum):
        # contract all four per-offset-pair partials directly: the whole
        # offset-sum tree is folded into the PSUM accumulation
        for o in range(2):
            yp = psum_y.tile([P, 512], f32, name="ypt", tag="ypt")
            for ch in range(2):
                wt, off = ws["wo", ch]
                for h in range(2):
                    for t in range(2):
                        nc.tensor.matmul(
                            yp[:],
                            lhsT=wt[:, off + o * P:off + (o + 1) * P],
                            rhs=usum[ch][h][:, t, :],
                            start=(ch == 0 and h == 0 and t == 0),
                            stop=(ch == 1 and h == 1 and t == 1))
            nc.scalar.activation(
                y_sb[o][:, p, :], yp[:], Act.Identity,
                bias=bias_sb[:, o * 4 + 3:o * 4 + 4])
            nc.gpsimd.dma_start(
                out=y_dram[o * P:(o + 1) * P, p * 512:(p + 1) * 512],
                in_=y_sb[o][:, p, :])

    prev = None
    for p in range(NPAIR):
        usum = {}
        for ch in range(2):
            sel = selmk[:, (p * 2 + ch) * P:(p * 2 + ch + 1) * P]
            uts = []
            for half in range(2):
                bc = psum_bc.tile([P, 2, 512], f32, name="bc", tag="bc")
                for j in range(2):
                    ci = half * 2 + j
                    nc.tensor.matmul(
                        bc[:, j, :], lhsT=sel, rhs=p_sb[:, ci, :],
                        start=True, stop=True)
                ut = utp.tile([P, 2, 512], bf16, name="ut", tag="ut")
                nc.vector.tensor_mul(
                    ut[:], bc[:], _shift_ap_v(vv[ch][:, p], half))
                uts.append(ut)
            usum[ch] = uts
        if prev is not None:
            emit_y(prev[0], prev[1])
        else:
            dummy_y()
        prev = (p, usum)
    emit_y(prev[0], prev[1])


def build_nc():
    from contextlib import ExitStack
    nc = bacc.Bacc(trn_type="TRN2", target_bir_lowering=False, debug=False)
    d = {}
    for name in ("xq", "xk", "xv"):
        d[name] = nc.dram_tensor(name, [DM, ROWS], bf16, kind="ExternalInput").ap()
    for name in ("wqk", "wvo"):
        d[name] = nc.dram_tensor(name, [DM, 2 * DM], bf16, kind="ExternalInput").ap()
    d["bias"] = nc.dram_tensor("bias", [P, 8], f32, kind="ExternalInput").ap()
    d["selkm"] = nc.dram_tensor("selkm", [P, 2 * 224], bf16, kind="ExternalInput").ap()
    d["selmk"] = nc.dram_tensor("selmk", [P, 8 * P], bf16, kind="ExternalInput").ap()
    y = nc.dram_tensor("y", [DM, ROWS], bf16, kind="ExternalOutput").ap()
    with tile.TileContext(nc) as tc:
        with ExitStack() as ctx:
            _emit(ctx, tc, nc, d, y)
    nc.compile()
    return nc


def _bf16(a):
    import ml_dtypes
    return np.ascontiguousarray(a, dtype=np.float32).astype(ml_dtypes.bfloat16)


def make_shared_inputs(Wq, bq, Wk, bk, Wv, bv, Wo, bo):
    Wq, bq, Wk, bk, Wv, bv, Wo, bo = (
        np.asarray(a, np.float32)
        for a in (Wq, bq, Wk, bk, Wv, bv, Wo, bo))
    shared = {}
    shared["wqk"] = _bf16(np.concatenate([Wq.T, Wk.T], axis=1))
    shared["wvo"] = _bf16(np.concatenate([Wv.T, Wo.T], axis=1))
    # biases packed [128, 8]: col ch*4+j = (bq, bk, bv, bo)[j] of chip ch
    bias = np.zeros((P, 8), np.float32)
    for ch in range(2):
        for j, b in enumerate((bq, bk, bv, bo)):
            bias[:, ch * 4 + j] = b[ch * P:(ch + 1) * P]
    shared["bias"] = bias
    # selkm[d, ch*224 + 96+h] = 0.5 iff h == global head of feature
    # ch*128+d. The score matmul for pair p uses the column slice
    # [96-32p : 224-32p], whose column j = 32p+h lands the head-h sum on
    # psum partition 32p+h.
    selkm = np.zeros((P, 2 * 224), np.float32)
    for ch in range(2):
        for dd in range(P):
            selkm[dd, ch * 224 + 96 + ch * 4 + dd // 32] = SCALE
    shared["selkm"] = _bf16(selkm)
    # selmk column block b = p*2+ch holds a [128, 128] selector with
    # sel[32p + ch*4 + dd//32, dd] = 1, so lhsT = selmk[:, b*128:(b+1)*128]
    # broadcasts pair p's 8 head rows of p_sb onto chip ch's partitions.
    selmk = np.zeros((P, 8 * P), np.float32)
    for p in range(NPAIR):
        for ch in range(2):
            b = p * 2 + ch
            for dd in range(P):
                selmk[32 * p + ch * 4 + dd // 32, b * P + dd] = 1.0
    shared["selmk"] = _bf16(selmk)
    return shared


def make_core_inputs(query, key_in, value, core):
    # core i handles stock i: [4, 512, 256] -> feature-major [256, 2048]
    out = {}
    for name, x in (("xq", query), ("xk", key_in), ("xv", value)):
        xi = np.asarray(x[core], dtype=np.float32).reshape(ROWS, DM)
        out[name] = _bf16(xi.T)
    return out


def kernel(query, key_in, value, Wq, bq, Wk, bk, Wv, bv, Wo, bo):
    nc = build_nc()
    shared = make_shared_inputs(Wq, bq, Wk, bk, Wv, bv, Wo, bo)
    in_maps = []
    for core in range(NCORES):
        m = dict(shared)
        m.update(make_core_inputs(query, key_in, value, core))
        in_maps.append(m)
    res = run_bass_kernel_spmd(nc, in_maps, list(range(NCORES))).results
    # y is feature-major [256, 2048] bf16 -> [4, 512, 256] fp32
    y = np.stack([
        np.asarray(res[i]["y"], dtype=np.float32)
        .reshape(DM, NPAIR, 512).transpose(1, 2, 0)
        for i in range(NCORES)])
    return np.ascontiguousarray(y, dtype=np.float32)


# revision 44
# speedup vs baseline: 1.0342x; 1.0342x over previous
"""Trainium2 Bass kernel for sparse multi-headed attention (v3, bf16).

Semantics (verified against the reference):
  q = x_q @ Wq.T + bq (per head, dk=32), same for k, v
  for each row s: attend to keys {s-c : c in (5,3,1,0), c <= s}
    score_c[s] = q[s].k[s-c] / sqrt(4)
    p = softmax over valid offsets
    attn[s] = sum_c p_c[s] * v[s-c]
  y = attn @ Wo.T + bo

Sharding: data-parallel over d_stock (8 stocks -> 8 cores). Each core
processes 4 (stock,batch) pairs = 2048 rows. Weights replicated.

Device layout: feature-major bf16 activations [256 feats = 2 chips x 128,
rows]; fp32 PSUM accumulation. Scores/softmax live in a pair-block layout
[128 partitions = 4 pair-blocks x (8 heads + 24 unused), 4 offsets x 512].
k is stored as an alignment pair ([128, 2, 520]: row 0 = shifted by one
element, row 1 = plain) so every shifted bf16 read is 4-byte aligned and
the offset pairs (5,3) / (1,0) collapse into single strided DVE ops in
packed 2x mode. v is a single padded row ([128, 520]); its reads feed
1x PSUM-operand multiplies where alignment doesn't matter. Pair-boundary
pad columns are never consumed with nonzero probability (the score mask
zeroes those lanes), so they are just memset once — no tail writes.
Output is produced transposed (feature-major) with the bias applied
per-partition; the host transposes back.
"""

import numpy as np

from concourse import bacc, bass, mybir, tile
from concourse.bass_utils import run_bass_kernel_spmd

DS, NB, S, DM, H, DK = 8, 4, 512, 256, 8, 32
CONS = (5, 3, 1, 0)
NCORES = 8
NPAIR = NB  # pairs per core (1 stock x 4 batches)
ROWS = NPAIR * S  # 2048
P = 128
PADC = 8  # zero pad columns in front of k/v tiles
KW = PADC + S  # 520
NEG = -1e9
SCALE = 0.5  # 1/sqrt(n_att)

f32 = mybir.dt.float32
bf16 = mybir.dt.bfloat16
Act = mybir.ActivationFunctionType


def _shift_ap_k(t, step):
    """AP over a [P, 2, KW] k pair tile covering two shifted 512-reads.

    step=2  -> offsets (5,3): row 0 (shifted copy) at bases 4,6
    step=KW -> offsets (1,0): row 0 base 8, row 1 (plain) base 8
    All bases even -> 4-byte aligned -> DVE 2x packed mode.
    """
    tv = t[:].rearrange("a o k -> a (o k)")
    off = 4 if step == 2 else PADC
    sl = tv[:, off:off + S]
    return bass.AP(tensor=sl.tensor, offset=sl.offset,
                   ap=[sl.ap[0], [step, 2], [1, S]])


def _shift_ap_v(t, half):
    """AP over a [P, KW] v tile: half 0 -> offsets (5,3) at bases 3,5;
    half 1 -> offsets (1,0) at bases 7,8. Feeds 1x PSUM-operand muls."""
    off, step = (3, 2) if half == 0 else (7, 1)
    sl = t[:, off:off + S]
    return bass.AP(tensor=sl.tensor, offset=sl.offset,
                   ap=[sl.ap[0], [step, 2], [1, S]])


def _emit(ctx, tc, nc, d, y_dram):
    main = ctx.enter_context(tc.tile_pool(name="main", bufs=1))
    prodp = ctx.enter_context(tc.tile_pool(name="prodp", bufs=8))
    utp = ctx.enter_context(tc.tile_pool(name="utp", bufs=10))
    smx = ctx.enter_context(tc.tile_pool(name="smx", bufs=6))
    psum_dum = ctx.enter_context(tc.tile_pool(name="dum", bufs=1, space="PSUM"))
    pj_ctx = tc.tile_pool(name="pj", bufs=3, space="PSUM")
    psum_pj = pj_ctx.__enter__()
    sc_ctx = tc.tile_pool(name="sc", bufs=1, space="PSUM")
    psum_sc = sc_ctx.__enter__()
    sc = psum_sc.tile([P, 4 * 512], f32, name="scores")

    # ---------------- PE warmup (HAM un-throttle) while DMAs run ----
    wtile = main.tile([P, 512], bf16, name="wtile")
    nc.vector.memset(wtile[:], 0.0)

    def dummy_mm():
        wps = psum_dum.tile([P, 512], f32, name="wps", tag="dum")
        nc.tensor.matmul(
            wps[:], lhsT=wtile[:, 0:P], rhs=wtile[:], start=True, stop=True)

    for i in range(5):
        dummy_mm()

    # ---------------- loads (scalar queue carries NO DMAs) ----------
    xs = {}
    for name in ("xq", "xk", "xv"):
        for ch in range(2):
            xs[name, ch] = main.tile([P, ROWS], bf16, name=f"{name}{ch}")
    # packed weights: wqk[kch] = [wq | wk] cols, wvo[kch] = [wv | wo]
    wqk = []
    wvo = []
    for kch in range(2):
        t = main.tile([P, 2 * DM], bf16, name=f"wqk{kch}")
        eng = nc.sync if kch == 0 else nc.scalar
        eng.dma_start(out=t[:], in_=d["wqk"][kch * P:(kch + 1) * P, :])
        wqk.append(t)
        t = main.tile([P, 2 * DM], bf16, name=f"wvo{kch}")
        wvo.append(t)
    # (tile, col offset) for each packed weight
    ws = {}
    for kch in range(2):
        ws["wq", kch] = (wqk[kch], 0)
        ws["wk", kch] = (wqk[kch], DM)
        ws["wv", kch] = (wvo[kch], 0)
        ws["wo", kch] = (wvo[kch], DM)
    # biases packed [128, 8]: col ch*4+j = (bq, bk, bv, bo)[j] of chip ch
    bias_sb = main.tile([P, 8], f32, name="bias")
    nc.sync.dma_start(out=bias_sb[:], in_=d["bias"])
    selkm_sb = main.tile([P, 2 * 224], bf16, name="selkm")
    nc.sync.dma_start(out=selkm_sb[:], in_=d["selkm"])
    # critical first slices: small, issued first (on three separate
    # rings) so they get bandwidth ahead of the bulk
    for ch in range(2):
        eng = nc.sync if ch == 0 else nc.scalar
        eng.dma_start(out=xs["xq", ch][:, 0:512],
                      in_=d["xq"][ch * P:(ch + 1) * P, 0:512])
        nc.gpsimd.dma_start(out=xs["xk", ch][:, 0:512],
                            in_=d["xk"][ch * P:(ch + 1) * P, 0:512])
    selmk = main.tile([P, 8 * P], bf16, name="selmk")

    # ---------------- q/k/v projections (PE, bf16) ----------------
    # Per chip: q [128, NPAIR, 512]; k alignment-pair [128, NPAIR, 2, 520]
    # (row 0 = row 1 shifted one element); v padded row [128, NPAIR, 520].
    # Projections are emitted in pair-pairs: 4 matmuls into a 2-bank PSUM
    # tile drained by a single 1024-col scalar activation (halves the
    # activation count on the pacing scalar queue). Pad heads are memset
    # once -- masked lanes never consume them with nonzero probability.
    q = {}
    kvv = {}
    vv = {}
    for ch in range(2):
        q[ch] = main.tile([P, NPAIR, 512], bf16, name=f"q{ch}")
        kvv[ch] = main.tile([P, NPAIR, 2, KW], bf16, name=f"k{ch}")
        vv[ch] = main.tile([P, NPAIR, KW], bf16, name=f"v{ch}")
    # memsets sit between the critical and bulk DMA issues on the gpsimd
    # queue, giving the critical transfers a bandwidth head start
    for ch in range(2):
        for p in range(NPAIR):
            nc.gpsimd.memset(kvv[ch][:, p, 0, 0:PADC + 1], 0.0)
            nc.gpsimd.memset(kvv[ch][:, p, 1, 0:PADC], 0.0)
            nc.gpsimd.memset(vv[ch][:, p, 0:PADC], 0.0)
    for ch in range(2):
        nc.gpsimd.dma_start(out=xs["xq", ch][:, 512:2048],
                            in_=d["xq"][ch * P:(ch + 1) * P, 512:2048])
        nc.gpsimd.dma_start(out=xs["xk", ch][:, 512:2048],
                            in_=d["xk"][ch * P:(ch + 1) * P, 512:2048])
    for kch in range(2):
        nc.gpsimd.dma_start(out=wvo[kch][:],
                            in_=d["wvo"][kch * P:(kch + 1) * P, :])
    for ch in range(2):
        nc.gpsimd.dma_start(out=xs["xv", ch][:],
                            in_=d["xv"][ch * P:(ch + 1) * P, :])
    nc.gpsimd.dma_start(out=selmk[:], in_=d["selmk"])

    def project(name, wname, bcol, n, ch):
        ps = psum_pj.tile([P, 512], f32, name="pjt", tag="pjt")
        for kch in range(2):
            wt, off = ws[wname, kch]
            nc.tensor.matmul(
                ps[:],
                lhsT=wt[:, off + ch * P:off + (ch + 1) * P],
                rhs=xs[name, kch][:, n * 512:(n + 1) * 512],
                start=(kch == 0), stop=(kch == 1))
        bias_ap = bias_sb[:, ch * 4 + bcol:ch * 4 + bcol + 1]
        if name == "xq":
            nc.scalar.activation(
                q[ch][:, n, :], ps[:], Act.Identity, bias=bias_ap)
        elif name == "xk":
            t = kvv[ch]
            nc.scalar.activation(
                t[:, n, 1, PADC:KW], ps[:], Act.Identity, bias=bias_ap)
            # shifted row 0 = cheap DVE SBUF copy of row 1
            nc.vector.tensor_copy(
                t[:, n, 0, PADC + 1:KW], t[:, n, 1, PADC:KW - 1])
        else:
            nc.scalar.activation(
                vv[ch][:, n, PADC:KW], ps[:], Act.Identity, bias=bias_ap)

    for n in range(4):
        for ch in range(2):
            project("xq", "wq", 0, n, ch)
            project("xk", "wk", 1, n, ch)
    # v projections fill the PE gap while the first products run on DVE
    for n in range(4):
        for ch in range(2):
            project("xv", "wv", 2, n, ch)

    # ---------------- scores ----------------
    # sc[32*pair + h, ci*512 + s] = q_h[s] . k_h[s-c] * 0.5
    # Half-major order: regions (5,3) finish first so their mask + exp +
    # partial denominator overlap the (1,0) product/matmul batch.
    p_sb = main.tile([P, 4, 512], bf16, name="p_sb")
    d1 = smx.tile([P, 512], bf16, name="d1", tag="smx")
    for half, step in ((0, 2), (1, KW)):
        for p in range(NPAIR):
            for ch in range(2):
                q_b = q[ch][:, p, :].rearrange(
                    "a (o s) -> a o s", o=1).broadcast_to([P, 2, 512])
                pr = prodp.tile([P, 2, 512], bf16, name="prod", tag="prod")
                nc.vector.tensor_mul(
                    pr[:], q_b, _shift_ap_k(kvv[ch][:, p], step))
                for j in range(2):
                    ci = half * 2 + j
                    nc.tensor.matmul(
                        sc[:, ci * 512:(ci + 1) * 512],
                        lhsT=selkm_sb[:, ch * 224 + 96 - 32 * p:
                                      ch * 224 + 224 - 32 * p],
                        rhs=pr[:, j, :],
                        start=(p == 0 and ch == 0),
                        stop=(p == 3 and ch == 1))
        # mask scores for s_loc < c -> -1e9, then exp this half's regions
        for j in range(2):
            ci = half * 2 + j
            c = CONS[ci]
            if c:
                nc.vector.memset(sc[:, ci * 512: ci * 512 + c], NEG)
            nc.scalar.activation(
                p_sb[:, ci, :], sc[:, ci * 512:(ci + 1) * 512], Act.Exp)
        if half == 0:
            nc.vector.tensor_add(d1[:], p_sb[:, 0, :], p_sb[:, 1, :])

    # ---------------- softmax over the 4 offsets (no max-sub: scores
    # are O(15) and masked lanes exp to 0) ----------------
    dummy_mm()  # HAM keepalive across the softmax bridge
    d2 = smx.tile([P, 512], bf16, name="d2", tag="smx")
    nc.vector.tensor_add(d2[:], p_sb[:, 2, :], p_sb[:, 3, :])
    den = smx.tile([P, 512], f32, name="den", tag="smx")
    nc.vector.tensor_add(den[:], d1[:], d2[:])
    rcp = smx.tile([P, 512], f32, name="rcp", tag="smx")
    nc.vector.reciprocal_approx_fast(rcp[:], den[:])
    rcpb = smx.tile([P, 512], bf16, name="rcpb", tag="smx")
    nc.vector.tensor_copy(rcpb[:], rcp[:])
    dummy_mm()
    rcp_b = rcpb[:].rearrange("a (o s) -> a o s", o=1).broadcast_to(
        [P, 4, 512])
    nc.vector.tensor_mul(p_sb[:], p_sb[:], rcp_b)

    # free proj/score PSUM, open attention-phase pools
    sc_ctx.__exit__(None, None, None)
    pj_ctx.__exit__(None, None, None)
    psum_bc = ctx.enter_context(tc.tile_pool(name="bc", bufs=2, space="PSUM"))
    psum_y = ctx.enter_context(tc.tile_pool(name="yp", bufs=2, space="PSUM"))

    dummy_y = dummy_mm

    # ---------------- attention + output projection, per pair --------
    y_sb = [main.tile([P, NPAIR, 512], bf16, name=f"y{o}") for o in range(2)]

    def emit_y(p, usum):
        # contract all four per-offset-pair partials directly: the whole
        # offset-sum tree is folded into the PSUM accumulation
        for o in range(2):
            yp = psum_y.tile([P, 512], f32, name="ypt", tag="ypt")
            for ch in range(2):
                wt, off = ws["wo", ch]
                for h in range(2):
                    for t in range(2):
                        nc.tensor.matmul(
                            yp[:],
                            lhsT=wt[:, off + o * P:off + (o + 1) * P],
                            rhs=usum[ch][h][:, t, :],
                            start=(ch == 0 and h == 0 and t == 0),
                            stop=(ch == 1 and h == 1 and t == 1))
            nc.scalar.activation(
                y_sb[o][:, p, :], yp[:], Act.Identity,
                bias=bias_sb[:, o * 4 + 3:o * 4 + 4])
            nc.gpsimd.dma_start(
                out=y_dram[o * P:(o + 1) * P, p * 512:(p + 1) * 512],
                in_=y_sb[o][:, p, :])

    prev = None
    for p in range(NPAIR):
        usum = {}
        for ch in range(2):
            sel = selmk[:, (p * 2 + ch) * P:(p * 2 + ch + 1) * P]
            uts = []
            for half in range(2):
                bc = psum_bc.tile([P, 2, 512], f32, name="bc", tag="bc")
                for j in range(2):
                    ci = half * 2 + j
                    nc.tensor.matmul(
                        bc[:, j, :], lhsT=sel, rhs=p_sb[:, ci, :],
                        start=True, stop=True)
                ut = utp.tile([P, 2, 512], bf16, name="ut", tag="ut")
                nc.vector.tensor_mul(
                    ut[:], bc[:], _shift_ap_v(vv[ch][:, p], half))
                uts.append(ut)
            usum[ch] = uts
        if prev is not None:
            emit_y(prev[0], prev[1])
        else:
            dummy_y()
        prev = (p, usum)
    emit_y(prev[0], prev[1])


def build_nc():
    from contextlib import ExitStack
    nc = bacc.Bacc(trn_type="TRN2", target_bir_lowering=False, debug=False)
    d = {}
    for name in ("xq", "xk", "xv"):
        d[name] = nc.dram_tensor(name, [DM, ROWS], bf16, kind="ExternalInput").ap()
    for name in ("wqk", "wvo"):
        d[name] = nc.dram_tensor(name, [DM, 2 * DM], bf16, kind="ExternalInput").ap()
    d["bias"] = nc.dram_tensor("bias", [P, 8], f32, kind="ExternalInput").ap()
    d["selkm"] = nc.dram_tensor("selkm", [P, 2 * 224], bf16, kind="ExternalInput").ap()
    d["selmk"] = nc.dram_tensor("selmk", [P, 8 * P], bf16, kind="ExternalInput").ap()
    y = nc.dram_tensor("y", [DM, ROWS], bf16, kind="ExternalOutput").ap()
    with tile.TileContext(nc) as tc:
        with ExitStack() as ctx:
            _emit(ctx, tc, nc, d, y)
    nc.compile()
    return nc


def _bf16(a):
    import ml_dtypes
    return np.ascontiguousarray(a, dtype=np.float32).astype(ml_dtypes.bfloat16)


def make_shared_inputs(Wq, bq, Wk, bk, Wv, bv, Wo, bo):
    Wq, bq, Wk, bk, Wv, bv, Wo, bo = (
        np.asarray(a, np.float32)
        for a in (Wq, bq, Wk, bk, Wv, bv, Wo, bo))
    shared = {}
    shared["wqk"] = _bf16(np.concatenate([Wq.T, Wk.T], axis=1))
    shared["wvo"] = _bf16(np.concatenate([Wv.T, Wo.T], axis=1))
    # biases packed [128, 8]: col ch*4+j = (bq, bk, bv, bo)[j] of chip ch
    bias = np.zeros((P, 8), np.float32)
    for ch in range(2):
        for j, b in enumerate((bq, bk, bv, bo)):
            bias[:, ch * 4 + j] = b[ch * P:(ch + 1) * P]
    shared["bias"] = bias
    # selkm[d, ch*224 + 96+h] = 0.5 iff h == global head of feature
    # ch*128+d. The score matmul for pair p uses the column slice
    # [96-32p : 224-32p], whose column j = 32p+h lands the head-h sum on
    # psum partition 32p+h.
    selkm = np.zeros((P, 2 * 224), np.float32)
    for ch in range(2):
        for dd in range(P):
            selkm[dd, ch * 224 + 96 + ch * 4 + dd // 32] = SCALE
    shared["selkm"] = _bf16(selkm)
    # selmk column block b = p*2+ch holds a [128, 128] selector with
    # sel[32p + ch*4 + dd//32, dd] = 1, so lhsT = selmk[:, b*128:(b+1)*128]
    # broadcasts pair p's 8 head rows of p_sb onto chip ch's partitions.
    selmk = np.zeros((P, 8 * P), np.float32)
    for p in range(NPAIR):
        for ch in range(2):
            b = p * 2 + ch
            for dd in range(P):
                selmk[32 * p + ch * 4 + dd // 32, b * P + dd] = 1.0
    shared["selmk"] = _bf16(selmk)
    return shared


def make_core_inputs(query, key_in, value, core):
    # core i handles stock i: [4, 512, 256] -> feature-major [256, 2048]
    out = {}
    for name, x in (("xq", query), ("xk", key_in), ("xv", value)):
        xi = np.asarray(x[core], dtype=np.float32).reshape(ROWS, DM)
        out[name] = _bf16(xi.T)
    return out


def kernel(query, key_in, value, Wq, bq, Wk, bk, Wv, bv, Wo, bo):
    nc = build_nc()
    shared = make_shared_inputs(Wq, bq, Wk, bk, Wv, bv, Wo, bo)
    in_maps = []
    for core in range(NCORES):
        m = dict(shared)
        m.update(make_core_inputs(query, key_in, value, core))
        in_maps.append(m)
    res = run_bass_kernel_spmd(nc, in_maps, list(range(NCORES))).results
    # y is feature-major [256, 2048] bf16 -> [4, 512, 256] fp32
    y = np.stack([
        np.asarray(res[i]["y"], dtype=np.float32)
        .reshape(DM, NPAIR, 512).transpose(1, 2, 0)
        for i in range(NCORES)])
    return np.ascontiguousarray(y, dtype=np.float32)


# revision 46
# speedup vs baseline: 1.0346x; 1.0003x over previous
"""Trainium2 Bass kernel for sparse multi-headed attention (v3, bf16).

Semantics (verified against the reference):
  q = x_q @ Wq.T + bq (per head, dk=32), same for k, v
  for each row s: attend to keys {s-c : c in (5,3,1,0), c <= s}
    score_c[s] = q[s].k[s-c] / sqrt(4)
    p = softmax over valid offsets
    attn[s] = sum_c p_c[s] * v[s-c]
  y = attn @ Wo.T + bo

Sharding: data-parallel over d_stock (8 stocks -> 8 cores). Each core
processes 4 (stock,batch) pairs = 2048 rows. Weights replicated.

Device layout: feature-major bf16 activations [256 feats = 2 chips x 128,
rows]; fp32 PSUM accumulation. Scores/softmax live in a pair-block layout
[128 partitions = 4 pair-blocks x (8 heads + 24 unused), 4 offsets x 512].
k is stored as an alignment pair ([128, 2, 520]: row 0 = shifted by one
element, row 1 = plain) so every shifted bf16 read is 4-byte aligned and
the offset pairs (5,3) / (1,0) collapse into single strided DVE ops in
packed 2x mode. v is a single padded row ([128, 520]); its reads feed
1x PSUM-operand multiplies where alignment doesn't matter. Pair-boundary
pad columns are never consumed with nonzero probability (the score mask
zeroes those lanes), so they are just memset once — no tail writes.
Output is produced transposed (feature-major) with the bias applied
per-partition; the host transposes back.
"""

import numpy as np

from concourse import bacc, bass, mybir, tile
from concourse.bass_utils import run_bass_kernel_spmd

DS, NB, S, DM, H, DK = 8, 4, 512, 256, 8, 32
CONS = (5, 3, 1, 0)
NCORES = 8
NPAIR = NB  # pairs per core (1 stock x 4 batches)
ROWS = NPAIR * S  # 2048
P = 128
PADC = 8  # zero pad columns in front of k/v tiles
KW = PADC + S  # 520
NEG = -1e9
SCALE = 0.5  # 1/sqrt(n_att)

f32 = mybir.dt.float32
bf16 = mybir.dt.bfloat16
Act = mybir.ActivationFunctionType


def _shift_ap_k(t, step):
    """AP over a [P, 2, KW] k pair tile covering two shifted 512-reads.

    step=2  -> offsets (5,3): row 0 (shifted copy) at bases 4,6
    step=KW -> offsets (1,0): row 0 base 8, row 1 (plain) base 8
    All bases even -> 4-byte aligned -> DVE 2x packed mode.
    """
    tv = t[:].rearrange("a o k -> a (o k)")
    off = 4 if step == 2 else PADC
    sl = tv[:, off:off + S]
    return bass.AP(tensor=sl.tensor, offset=sl.offset,
                   ap=[sl.ap[0], [step, 2], [1, S]])


def _shift_ap_v(t, half):
    """AP over a [P, KW] v tile: half 0 -> offsets (5,3) at bases 3,5;
    half 1 -> offsets (1,0) at bases 7,8. Feeds 1x PSUM-operand muls."""
    off, step = (3, 2) if half == 0 else (7, 1)
    sl = t[:, off:off + S]
    return bass.AP(tensor=sl.tensor, offset=sl.offset,
                   ap=[sl.ap[0], [step, 2], [1, S]])


def _emit(ctx, tc, nc, d, y_dram):
    main = ctx.enter_context(tc.tile_pool(name="main", bufs=1))
    prodp = ctx.enter_context(tc.tile_pool(name="prodp", bufs=8))
    utp = ctx.enter_context(tc.tile_pool(name="utp", bufs=10))
    smx = ctx.enter_context(tc.tile_pool(name="smx", bufs=6))
    psum_dum = ctx.enter_context(tc.tile_pool(name="dum", bufs=1, space="PSUM"))
    pj_ctx = tc.tile_pool(name="pj", bufs=3, space="PSUM")
    psum_pj = pj_ctx.__enter__()
    sc_ctx = tc.tile_pool(name="sc", bufs=1, space="PSUM")
    psum_sc = sc_ctx.__enter__()
    sc = psum_sc.tile([P, 4 * 512], f32, name="scores")

    # ---------------- PE warmup (HAM un-throttle) while DMAs run ----
    wtile = main.tile([P, 512], bf16, name="wtile")
    nc.vector.memset(wtile[:], 0.0)

    def dummy_mm():
        wps = psum_dum.tile([P, 512], f32, name="wps", tag="dum")
        nc.tensor.matmul(
            wps[:], lhsT=wtile[:, 0:P], rhs=wtile[:], start=True, stop=True)

    for i in range(5):
        dummy_mm()

    # ---------------- loads (scalar queue carries NO DMAs) ----------
    xs = {}
    for name in ("xq", "xk", "xv"):
        for ch in range(2):
            xs[name, ch] = main.tile([P, ROWS], bf16, name=f"{name}{ch}")
    # packed weights: wqk[kch] = [wq | wk] cols, wvo[kch] = [wv | wo]
    wqk = []
    wvo = []
    for kch in range(2):
        t = main.tile([P, 2 * DM], bf16, name=f"wqk{kch}")
        eng = nc.sync if kch == 0 else nc.scalar
        eng.dma_start(out=t[:], in_=d["wqk"][kch * P:(kch + 1) * P, :])
        wqk.append(t)
        t = main.tile([P, 2 * DM], bf16, name=f"wvo{kch}")
        wvo.append(t)
    # (tile, col offset) for each packed weight
    ws = {}
    for kch in range(2):
        ws["wq", kch] = (wqk[kch], 0)
        ws["wk", kch] = (wqk[kch], DM)
        ws["wv", kch] = (wvo[kch], 0)
        ws["wo", kch] = (wvo[kch], DM)
    # biases packed [128, 8]: col ch*4+j = (bq, bk, bv, bo)[j] of chip ch
    bias_sb = main.tile([P, 8], f32, name="bias")
    nc.sync.dma_start(out=bias_sb[:], in_=d["bias"])
    selkm_sb = main.tile([P, 2 * 224], bf16, name="selkm")
    nc.sync.dma_start(out=selkm_sb[:], in_=d["selkm"])
    # critical first slices: small, issued first (on three separate
    # rings) so they get bandwidth ahead of the bulk
    for ch in range(2):
        eng = nc.sync if ch == 0 else nc.scalar
        eng.dma_start(out=xs["xq", ch][:, 0:512],
                      in_=d["xq"][ch * P:(ch + 1) * P, 0:512])
        nc.gpsimd.dma_start(out=xs["xk", ch][:, 0:512],
                            in_=d["xk"][ch * P:(ch + 1) * P, 0:512])
    selmk = main.tile([P, 8 * P], bf16, name="selmk")

    # ---------------- q/k/v projections (PE, bf16) ----------------
    # Per chip: q [128, NPAIR, 512]; k alignment-pair [128, NPAIR, 2, 520]
    # (row 0 = row 1 shifted one element); v padded row [128, NPAIR, 520].
    # Projections are emitted in pair-pairs: 4 matmuls into a 2-bank PSUM
    # tile drained by a single 1024-col scalar activation (halves the
    # activation count on the pacing scalar queue). Pad heads are memset
    # once -- masked lanes never consume them with nonzero probability.
    q = {}
    kvv = {}
    vv = {}
    for ch in range(2):
        q[ch] = main.tile([P, NPAIR, 512], bf16, name=f"q{ch}")
        kvv[ch] = main.tile([P, NPAIR, 2, KW], bf16, name=f"k{ch}")
        vv[ch] = main.tile([P, NPAIR, KW], bf16, name=f"v{ch}")
    # memsets sit between the critical and bulk DMA issues on the gpsimd
    # queue, giving the critical transfers a bandwidth head start
    for ch in range(2):
        for p in range(NPAIR):
            nc.gpsimd.memset(kvv[ch][:, p, 0, 0:PADC + 1], 0.0)
            nc.gpsimd.memset(kvv[ch][:, p, 1, 0:PADC], 0.0)
            nc.gpsimd.memset(vv[ch][:, p, 0:PADC], 0.0)
    for ch in range(2):
        nc.gpsimd.dma_start(out=xs["xq", ch][:, 512:2048],
                            in_=d["xq"][ch * P:(ch + 1) * P, 512:2048])
        nc.gpsimd.dma_start(out=xs["xk", ch][:, 512:2048],
                            in_=d["xk"][ch * P:(ch + 1) * P, 512:2048])
    for kch in range(2):
        nc.gpsimd.dma_start(out=wvo[kch][:],
                            in_=d["wvo"][kch * P:(kch + 1) * P, :])
    for ch in range(2):
        nc.gpsimd.dma_start(out=xs["xv", ch][:],
                            in_=d["xv"][ch * P:(ch + 1) * P, :])
    nc.gpsimd.dma_start(out=selmk[:], in_=d["selmk"])

    def project(name, wname, bcol, n, ch):
        ps = psum_pj.tile([P, 512], f32, name="pjt", tag="pjt")
        for kch in range(2):
            wt, off = ws[wname, kch]
            nc.tensor.matmul(
                ps[:],
                lhsT=wt[:, off + ch * P:off + (ch + 1) * P],
                rhs=xs[name, kch][:, n * 512:(n + 1) * 512],
                start=(kch == 0), stop=(kch == 1))
        bias_ap = bias_sb[:, ch * 4 + bcol:ch * 4 + bcol + 1]
        if name == "xq":
            nc.scalar.activation(
                q[ch][:, n, :], ps[:], Act.Identity, bias=bias_ap)
        elif name == "xk":
            t = kvv[ch]
            nc.scalar.activation(
                t[:, n, 1, PADC:KW], ps[:], Act.Identity, bias=bias_ap)
            # shifted row 0 = cheap DVE SBUF copy of row 1
            nc.vector.tensor_copy(
                t[:, n, 0, PADC + 1:KW], t[:, n, 1, PADC:KW - 1])
        else:
            nc.scalar.activation(
                vv[ch][:, n, PADC:KW], ps[:], Act.Identity, bias=bias_ap)

    for n in range(4):
        for ch in range(2):
            project("xq", "wq", 0, n, ch)
            project("xk", "wk", 1, n, ch)
    # v projections fill the PE gap while the first products run on DVE
    for n in range(4):
        for ch in range(2):
            project("xv", "wv", 2, n, ch)

    # ---------------- scores ----------------
    # sc[32*pair + h, ci*512 + s] = q_h[s] . k_h[s-c] * 0.5
    # Half-major order: regions (5,3) finish first so their mask + exp +
    # partial denominator overlap the (1,0) product/matmul batch.
    p_sb = main.tile([P, 4, 512], bf16, name="p_sb")
    d1 = smx.tile([P, 512], bf16, name="d1", tag="smx")
    for half, step in ((0, 2), (1, KW)):
        for p in range(NPAIR):
            for ch in range(2):
                q_b = q[ch][:, p, :].rearrange(
                    "a (o s) -> a o s", o=1).broadcast_to([P, 2, 512])
                pr = prodp.tile([P, 2, 512], bf16, name="prod", tag="prod")
                nc.vector.tensor_mul(
                    pr[:], q_b, _shift_ap_k(kvv[ch][:, p], step))
                for j in range(2):
                    ci = half * 2 + j
                    nc.tensor.matmul(
                        sc[:, ci * 512:(ci + 1) * 512],
                        lhsT=selkm_sb[:, ch * 224 + 96 - 32 * p:
                                      ch * 224 + 224 - 32 * p],
                        rhs=pr[:, j, :],
                        start=(p == 0 and ch == 0),
                        stop=(p == 3 and ch == 1))
        # mask scores for s_loc < c -> -1e9, then exp this half's regions
        for j in range(2):
            ci = half * 2 + j
            c = CONS[ci]
            if c:
                nc.vector.memset(sc[:, ci * 512: ci * 512 + c], NEG)
            nc.scalar.activation(
                p_sb[:, ci, :], sc[:, ci * 512:(ci + 1) * 512], Act.Exp)
        if half == 0:
            nc.vector.tensor_add(d1[:], p_sb[:, 0, :], p_sb[:, 1, :])

    # ---------------- softmax over the 4 offsets (no max-sub: scores
    # are O(15) and masked lanes exp to 0) ----------------
    dummy_mm()  # HAM keepalive across the softmax bridge
    d2 = smx.tile([P, 512], bf16, name="d2", tag="smx")
    nc.vector.tensor_add(d2[:], p_sb[:, 2, :], p_sb[:, 3, :])
    den = smx.tile([P, 512], f32, name="den", tag="smx")
    nc.vector.tensor_add(den[:], d1[:], d2[:])
    rcp = smx.tile([P, 512], f32, name="rcp", tag="smx")
    nc.vector.reciprocal_approx_fast(rcp[:], den[:])
    rcpb = smx.tile([P, 512], bf16, name="rcpb", tag="smx")
    nc.vector.tensor_copy(rcpb[:], rcp[:])
    dummy_mm()
    rcp_b = rcpb[:].rearrange("a (o s) -> a o s", o=1).broadcast_to(
        [P, 2, 512])
    # normalize per half so the first bc broadcast starts half-early
    nc.vector.tensor_mul(p_sb[:, 0:2, :], p_sb[:, 0:2, :], rcp_b)
    nc.vector.tensor_mul(p_sb[:, 2:4, :], p_sb[:, 2:4, :], rcp_b)

    # free proj/score PSUM, open attention-phase pools
    sc_ctx.__exit__(None, None, None)
    pj_ctx.__exit__(None, None, None)
    psum_bc = ctx.enter_context(tc.tile_pool(name="bc", bufs=2, space="PSUM"))
    psum_y = ctx.enter_context(tc.tile_pool(name="yp", bufs=2, space="PSUM"))

    dummy_y = dummy_mm

    # ---------------- attention + output projection, per pair --------
    y_sb = [main.tile([P, NPAIR, 512], bf16, name=f"y{o}") for o in range(2)]

    def y_chunks(p, usum):
        # the whole offset-sum tree is folded into the output-projection
        # PSUM accumulation; emitted as four 4-matmul thunks so they can
        # interleave between the next pair's bc groups (keeps PE feeding
        # DVE with fresh bc tiles instead of running one 16-matmul block)
        yps = {}

        def mk(o, cch):
            def go():
                if cch == 0:
                    yps[o] = psum_y.tile([P, 512], f32, name="ypt", tag="ypt")
                yp = yps[o]
                wt, off = ws["wo", cch]
                for h in range(2):
                    for t in range(2):
                        nc.tensor.matmul(
                            yp[:],
                            lhsT=wt[:, off + o * P:off + (o + 1) * P],
                            rhs=usum[cch][h][:, t, :],
                            start=(cch == 0 and h == 0 and t == 0),
                            stop=(cch == 1 and h == 1 and t == 1))
                if cch == 1:
                    nc.scalar.activation(
                        y_sb[o][:, p, :], yp[:], Act.Identity,
                        bias=bias_sb[:, o * 4 + 3:o * 4 + 4])
                    nc.gpsimd.dma_start(
                        out=y_dram[o * P:(o + 1) * P, p * 512:(p + 1) * 512],
                        in_=y_sb[o][:, p, :])
            return go
        return [mk(0, 0), mk(0, 1), mk(1, 0), mk(1, 1)]

    pending = []
    for p in range(NPAIR):
        usum = {}
        for ch in range(2):
            sel = selmk[:, (p * 2 + ch) * P:(p * 2 + ch + 1) * P]
            uts = []
            for half in range(2):
                bc = psum_bc.tile([P, 2, 512], f32, name="bc", tag="bc")
                for j in range(2):
                    ci = half * 2 + j
                    nc.tensor.matmul(
                        bc[:, j, :], lhsT=sel, rhs=p_sb[:, ci, :],
                        start=True, stop=True)
                if pending:
                    pending.pop(0)()
                elif p == 0 and ch == 0 and half == 0:
                    dummy_y()
                ut = utp.tile([P, 2, 512], bf16, name="ut", tag="ut")
                nc.vector.tensor_mul(
                    ut[:], bc[:], _shift_ap_v(vv[ch][:, p], half))
                uts.append(ut)
            usum[ch] = uts
        pending = y_chunks(p, usum)
    for f in pending:
        f()


def build_nc():
    from contextlib import ExitStack
    nc = bacc.Bacc(trn_type="TRN2", target_bir_lowering=False, debug=False)
    d = {}
    for name in ("xq", "xk", "xv"):
        d[name] = nc.dram_tensor(name, [DM, ROWS], bf16, kind="ExternalInput").ap()
    for name in ("wqk", "wvo"):
        d[name] = nc.dram_tensor(name, [DM, 2 * DM], bf16, kind="ExternalInput").ap()
    d["bias"] = nc.dram_tensor("bias", [P, 8], f32, kind="ExternalInput").ap()
    d["selkm"] = nc.dram_tensor("selkm", [P, 2 * 224], bf16, kind="ExternalInput").ap()
    d["selmk"] = nc.dram_tensor("selmk", [P, 8 * P], bf16, kind="ExternalInput").ap()
    y = nc.dram_tensor("y", [DM, ROWS], bf16, kind="ExternalOutput").ap()
    with tile.TileContext(nc) as tc:
        with ExitStack() as ctx:
            _emit(ctx, tc, nc, d, y)
    nc.compile()
    return nc


def _bf16(a):
    import ml_dtypes
    return np.ascontiguousarray(a, dtype=np.float32).astype(ml_dtypes.bfloat16)


def make_shared_inputs(Wq, bq, Wk, bk, Wv, bv, Wo, bo):
    Wq, bq, Wk, bk, Wv, bv, Wo, bo = (
        np.asarray(a, np.float32)
        for a in (Wq, bq, Wk, bk, Wv, bv, Wo, bo))
    shared = {}
    shared["wqk"] = _bf16(np.concatenate([Wq.T, Wk.T], axis=1))
    shared["wvo"] = _bf16(np.concatenate([Wv.T, Wo.T], axis=1))
    # biases packed [128, 8]: col ch*4+j = (bq, bk, bv, bo)[j] of chip ch
    bias = np.zeros((P, 8), np.float32)
    for ch in range(2):
        for j, b in enumerate((bq, bk, bv, bo)):
            bias[:, ch * 4 + j] = b[ch * P:(ch + 1) * P]
    shared["bias"] = bias
    # selkm[d, ch*224 + 96+h] = 0.5 iff h == global head of feature
    # ch*128+d. The score matmul for pair p uses the column slice
    # [96-32p : 224-32p], whose column j = 32p+h lands the head-h sum on
    # psum partition 32p+h.
    selkm = np.zeros((P, 2 * 224), np.float32)
    for ch in range(2):
        for dd in range(P):
            selkm[dd, ch * 224 + 96 + ch * 4 + dd // 32] = SCALE
    shared["selkm"] = _bf16(selkm)
    # selmk column block b = p*2+ch holds a [128, 128] selector with
    # sel[32p + ch*4 + dd//32, dd] = 1, so lhsT = selmk[:, b*128:(b+1)*128]
    # broadcasts pair p's 8 head rows of p_sb onto chip ch's partitions.
    selmk = np.zeros((P, 8 * P), np.float32)
    for p in range(NPAIR):
        for ch in range(2):
            b = p * 2 + ch
            for dd in range(P):
                selmk[32 * p + ch * 4 + dd // 32, b * P + dd] = 1.0
    shared["selmk"] = _bf16(selmk)
    return shared


def make_core_inputs(query, key_in, value, core):
    # core i handles stock i: [4, 512, 256] -> feature-major [256, 2048]
    out = {}
    for name, x in (("xq", query), ("xk", key_in), ("xv", value)):
        xi = np.asarray(x[core], dtype=np.float32).reshape(ROWS, DM)
        out[name] = _bf16(xi.T)
    return out


def kernel(query, key_in, value, Wq, bq, Wk, bk, Wv, bv, Wo, bo):
    nc = build_nc()
    shared = make_shared_inputs(Wq, bq, Wk, bk, Wv, bv, Wo, bo)
    in_maps = []
    for core in range(NCORES):
        m = dict(shared)
        m.update(make_core_inputs(query, key_in, value, core))
        in_maps.append(m)
    res = run_bass_kernel_spmd(nc, in_maps, list(range(NCORES))).results
    # y is feature-major [256, 2048] bf16 -> [4, 512, 256] fp32
    y = np.stack([
        np.asarray(res[i]["y"], dtype=np.float32)
        .reshape(DM, NPAIR, 512).transpose(1, 2, 0)
        for i in range(NCORES)])
    return np.ascontiguousarray(y, dtype=np.float32)
